# revision 28
# baseline (speedup 1.0000x reference)
"""Trainium2 Bass kernel: one dense transformer block on 8 NeuronCores.

Sequence-parallel: 8 cores = 2 batches x 4 chunks of 512 tokens. Each core
computes LN1+QKV for its chunk, AllGathers K/V within its 4-core batch group,
runs causal attention for its rows (k-major scores, ones-column softmax
denominator, structural masking), then attn-proj, LN2 and the MLP for its
rows. Host only shards inputs and concatenates the 8 output shards.

build_nc(iters=k) unrolls the block k times inside one NEFF (same inputs ->
same outputs each pass); the timing harness uses marginal time between a
k-iter and a 1-iter launch to cancel launch/dispatch overhead.
"""
import sys
for _p in ("/opt/trn_rl_repo", "/root/.axon_site/_ro/trn_rl_repo"):
    if _p not in sys.path:
        sys.path.append(_p)

from contextlib import ExitStack

import numpy as np

import concourse.bass as bass
import concourse.tile as tile
from concourse import bacc, mybir
from concourse.masks import make_identity

F32 = mybir.dt.float32
BF16 = mybir.dt.bfloat16

B, T, C, H, HD = 2, 2048, 1024, 16, 64
R = 512              # token rows per core
QT = R // 128        # 4 q-tiles of 128
FT = C // 128        # 8 feature tiles
NSLOT = T // 128     # 16 gathered k slots
VAW = H * (HD + 1)   # 1040: V_aug row width (64 V cols + 1 ones col per head)
GROUPS = [[0, 1, 2, 3], [4, 5, 6, 7]]
EPS = 1e-5

# Gathered-slot set: core j (j = chunk index within its batch group) attends
# gathered slots s < 4j; since j <= 3 only slots 0..11 are ever needed (each
# core's own 4 slots come from the in-SBUF own-diagonal pass). vzero masks
# per-core which of the 12 are live.
NGATH = 12
# slot groupings for the scores PSUM tile [128, 1024] (2 banks)
GATH_GROUPS = [(0, 1), (2, 3), (4, 5), (6, 7), (8, 9), (10, 11)]
# own-pass ragged spans: slot s covers queries q >= 128*s -> N = 512-128*s.
# Offsets packed so no matmul output straddles a 2KB PSUM bank:
# s0 [0,512) s1 [512,896) s3 [896,1024) s2 [1024,1280) -- contiguous 1280 f32.
OWN_OFF = [0, 512, 1024, 896]
OWN_N = [512, 384, 256, 128]
OWN_W = 1280


def build_nc(iters=1):
    nc = bacc.Bacc(None, num_devices=8)

    x_in = nc.dram_tensor("x", [R, C], F32, kind="ExternalInput")
    wqk = nc.dram_tensor("w_qk", [C, 2 * C], BF16, kind="ExternalInput")
    bqk = nc.dram_tensor("b_qk", [2 * C], F32, kind="ExternalInput")
    wv = nc.dram_tensor("w_v", [C, C], BF16, kind="ExternalInput")
    bv = nc.dram_tensor("b_v", [C], F32, kind="ExternalInput")
    wap = nc.dram_tensor("w_ap", [C, C], BF16, kind="ExternalInput")
    bap = nc.dram_tensor("b_ap", [C], F32, kind="ExternalInput")
    wfc = nc.dram_tensor("w_fc", [C, 4 * C], BF16, kind="ExternalInput")
    bfc = nc.dram_tensor("b_fc", [4 * C], F32, kind="ExternalInput")
    wmp = nc.dram_tensor("w_mp", [4 * C, C], BF16, kind="ExternalInput")
    bmp = nc.dram_tensor("b_mp", [C], F32, kind="ExternalInput")
    ln1g = nc.dram_tensor("ln1_g", [C], F32, kind="ExternalInput")
    ln1b = nc.dram_tensor("ln1_b", [C], F32, kind="ExternalInput")
    ln2g = nc.dram_tensor("ln2_g", [C], F32, kind="ExternalInput")
    ln2b = nc.dram_tensor("ln2_b", [C], F32, kind="ExternalInput")
    vzero = nc.dram_tensor("vzero", [NGATH], F32, kind="ExternalInput")
    out = nc.dram_tensor("out", [R, C], F32, kind="ExternalOutput")

    # K^T is AllGathered in two feature halves (heads 0-7 / 8-15) so the
    # gathered pass can start on the first half while the second still flies.
    HC = C // 2
    kt_sends_a = [nc.dram_tensor(f"kt_senda{i}", [HC, R], BF16) for i in range(iters)]
    kt_gaths_a = [nc.dram_tensor(f"kt_gatha{i}", [4 * HC, R], BF16) for i in range(iters)]
    kt_sends_b = [nc.dram_tensor(f"kt_sendb{i}", [HC, R], BF16) for i in range(iters)]
    kt_gaths_b = [nc.dram_tensor(f"kt_gathb{i}", [4 * HC, R], BF16) for i in range(iters)]
    v_sends = [nc.dram_tensor(f"v_send{i}", [R, VAW], BF16) for i in range(iters)]
    v_gaths = [nc.dram_tensor(f"v_gath{i}", [T, VAW], BF16) for i in range(iters)]

    def bcast(t, n):
        return bass.AP(tensor=t.tensor, offset=0, ap=[[0, 128], [1, n]])

    with tile.TileContext(nc) as tc, ExitStack() as top:
        singles = top.enter_context(tc.tile_pool(name="singles", bufs=1))

        ident = singles.tile([128, 128], BF16)
        make_identity(nc, ident)
        eps_t = singles.tile([128, 1], F32)
        nc.vector.memset(eps_t, EPS)
        ones64 = singles.tile([1, 64], BF16)
        nc.vector.memset(ones64, 1.0)

        def bcast_tile(src, n, dtype=BF16, name="", pool=None):
            t = (pool or singles).tile([128, n], dtype, tag=f"bc_{name}")
            eng = nc.gpsimd if dtype != F32 else nc.sync
            eng.dma_start(out=t, in_=bcast(src[:], n))
            return t

        vz_bc = bcast_tile(vzero, NGATH, F32, name="vz")

        bqk_sb = singles.tile([128, 16], F32)
        nc.sync.dma_start(out=bqk_sb, in_=bqk[:].rearrange("(mt p) -> p mt", p=128))
        bfc_sb = singles.tile([128, 32], F32)
        nc.sync.dma_start(out=bfc_sb, in_=bfc[:].rearrange("(mt p) -> p mt", p=128))
        bmp_sb = singles.tile([128, 8], F32)
        nc.sync.dma_start(out=bmp_sb, in_=bmp[:].rearrange("(mt p) -> p mt", p=128))

        # persistent activations (shared across unrolled iterations)
        x_sb = singles.tile([128, QT, C], F32)          # x, later x2 (residual)
        qT_sb = singles.tile([128, FT, R], BF16)        # Q^T feature-major
        kT_sb = singles.tile([128, FT, R], BF16)        # own K^T feature-major
        vaug_sb = singles.tile([128, QT, VAW], BF16)    # own V_aug token-major
        xnT = singles.tile([128, FT, R], BF16)          # LN1 out / later y^T

        # ---- LN helper: token-major x[:, qt, :] f32 -> bf16 normalized ----
        def layernorm(pool, src_qt):
            stats = pool.tile([128, 2, 6], F32, tag="ln_stats")
            mv = pool.tile([128, 2], F32, tag="ln_mv")
            for sg in range(2):
                nc.vector.bn_stats(out=stats[:, sg, :], in_=src_qt[:, 512 * sg : 512 * (sg + 1)])
            nc.vector.bn_aggr(out=mv, in_=stats)
            nc.scalar.activation(
                out=mv[:, 1:2], in_=mv[:, 1:2],
                func=mybir.ActivationFunctionType.Sqrt, bias=eps_t, scale=1.0,
            )
            nc.vector.reciprocal(out=mv[:, 1:2], in_=mv[:, 1:2])
            xn = pool.tile([128, C], BF16, tag="ln_xn")
            nc.vector.tensor_scalar(
                out=xn, in0=src_qt, scalar1=mv[:, 0:1], scalar2=mv[:, 1:2],
                op0=mybir.AluOpType.subtract, op1=mybir.AluOpType.mult,
            )
            return xn

        # transpose token-major bf16 [128, C] tile (one qt) into dst [128, FT, R];
        # optionally applies per-feature scale/bias (features = partitions here)
        def transpose_qt(pool, psum_pool, xn, dst, qt, gT=None, bT=None):
            for ft in range(FT):
                ps = psum_pool.tile([128, 128], BF16, tag="tr_ps")
                nc.tensor.transpose(ps, xn[:, 128 * ft : 128 * (ft + 1)], ident)
                if gT is None:
                    nc.vector.tensor_copy(out=dst[:, ft, 128 * qt : 128 * (qt + 1)], in_=ps)
                else:
                    nc.vector.tensor_scalar(
                        out=dst[:, ft, 128 * qt : 128 * (qt + 1)], in0=ps,
                        scalar1=gT[:, ft : ft + 1], scalar2=bT[:, ft : ft + 1],
                        op0=mybir.AluOpType.mult, op1=mybir.AluOpType.add,
                    )

        def body(it):
            kt_send_a, kt_gath_a = kt_sends_a[it], kt_gaths_a[it]
            kt_send_b, kt_gath_b = kt_sends_b[it], kt_gaths_b[it]
            v_send, v_gath = v_sends[it], v_gaths[it]

            for qt in range(QT):
                nc.sync.dma_start(
                    out=x_sb[:, qt, :],
                    in_=x_in[:].rearrange("(qt p) c -> p qt c", p=128)[:, qt, :],
                )

            # ================= LN1 + transpose =================
            with tc.tile_pool(name="ln1", bufs=3) as pool, \
                 tc.tile_pool(name="tr1_ps", bufs=2, space="PSUM") as psp:
                ln1gT = pool.tile([128, FT], F32, tag="ln1gT")
                nc.sync.dma_start(out=ln1gT, in_=ln1g[:].rearrange("(ft p) -> p ft", p=128))
                ln1bT = pool.tile([128, FT], F32, tag="ln1bT")
                nc.sync.dma_start(out=ln1bT, in_=ln1b[:].rearrange("(ft p) -> p ft", p=128))
                for qt in range(QT):
                    xn = layernorm(pool, x_sb[:, qt, :])
                    transpose_qt(pool, psp, xn, xnT, qt, gT=ln1gT, bT=ln1bT)

            # ================= QKV projections =================
            # K first -> AllGather K; V -> AllGather V; Q last (the attention
            # own-diagonal pass then covers the collective latency).
            with tc.tile_pool(name="wk_pool", bufs=3) as wpool, \
                 tc.tile_pool(name="qk_ps", bufs=4, space="PSUM") as psp:
                wts = {}
                for mt in range(8, 16):
                    if mt % 2 == 0:
                        wt2 = wpool.tile([128, FT, 256], BF16, tag="wqk")
                        nc.sync.dma_start(
                            out=wt2,
                            in_=wqk[:, 128 * mt : 128 * (mt + 2)].rearrange(
                                "(kt p) m -> p kt m", p=128
                            ),
                        )
                        wts[mt // 2] = wt2
                    wt = wts[mt // 2]
                    mo = 128 * (mt % 2)
                    ps = psp.tile([128, R], F32, tag="qk_ps")
                    for kt in range(FT):
                        nc.tensor.matmul(
                            ps, lhsT=wt[:, kt, mo : mo + 128], rhs=xnT[:, kt, :],
                            start=(kt == 0), stop=(kt == FT - 1),
                        )
                    nc.vector.tensor_scalar_add(
                        out=kT_sb[:, mt - 8, :], in0=ps,
                        scalar1=bqk_sb[:, mt : mt + 1],
                    )
                    if mt == 11:
                        # first K half (head features 0..3) -> AllGather now
                        nc.sync.dma_start(
                            out=kt_send_a[:].rearrange("(ft p) q -> p ft q", p=128),
                            in_=kT_sb[:, 0:4, :],
                        )
                        nc.gpsimd.collective_compute(
                            "AllGather", mybir.AluOpType.bypass,
                            replica_groups=GROUPS,
                            ins=[kt_send_a[:]], outs=[kt_gath_a[:]],
                        )
                # second K half: stage the send now; its collective is issued
                # after AllGather(V) so the queue order matches consumption
                # (kst_a -> vst -> kst_b).
                nc.sync.dma_start(
                    out=kt_send_b[:].rearrange("(ft p) q -> p ft q", p=128),
                    in_=kT_sb[:, 4:8, :],
                )

            with tc.tile_pool(name="wv_pool", bufs=1) as wpool, \
                 tc.tile_pool(name="v_ps", bufs=2, space="PSUM") as psp:
                wv_sb = wpool.tile([128, FT, C], BF16)
                for kt in range(FT):
                    nc.sync.dma_start(
                        out=wv_sb[:, kt, :],
                        in_=wv[128 * kt : 128 * (kt + 1), :],
                    )
                bv_bc = bcast_tile(bv, C, name="bv", pool=wpool)
                for qt in range(QT):
                    ps = psp.tile([128, C], F32, tag="v_ps")
                    for kt in range(FT):
                        for half in range(2):
                            nc.tensor.matmul(
                                ps[:, 512 * half : 512 * (half + 1)],
                                lhsT=xnT[:, kt, 128 * qt : 128 * (qt + 1)],
                                rhs=wv_sb[:, kt, 512 * half : 512 * (half + 1)],
                                start=(kt == 0), stop=(kt == FT - 1),
                            )
                    # v_aug[:, qt, h*65 : h*65+64] = psum[:, h*64:...] + bv
                    nc.vector.tensor_tensor(
                        out=vaug_sb[:, qt, :].rearrange("p (h w) -> p h w", h=H)[:, :, 0:HD],
                        in0=ps.rearrange("p (h w) -> p h w", h=H),
                        in1=bv_bc.rearrange("p (h w) -> p h w", h=H),
                        op=mybir.AluOpType.add,
                    )
                ones_view = vaug_sb.rearrange("p qt (h w) -> p qt h w", h=H)[:, :, :, HD : HD + 1]
                nc.vector.memset(ones_view, 1.0)
                nc.sync.dma_start(
                    out=v_send[:].rearrange("(qt p) w -> p qt w", p=128), in_=vaug_sb
                )
                nc.gpsimd.collective_compute(
                    "AllGather", mybir.AluOpType.bypass,
                    replica_groups=GROUPS,
                    ins=[v_send[:]], outs=[v_gath[:]],
                )
                nc.gpsimd.collective_compute(
                    "AllGather", mybir.AluOpType.bypass,
                    replica_groups=GROUPS,
                    ins=[kt_send_b[:]], outs=[kt_gath_b[:]],
                )

            with tc.tile_pool(name="wq_pool", bufs=3) as wpool, \
                 tc.tile_pool(name="q_ps", bufs=4, space="PSUM") as psp:
                wts = {}
                for mt in range(8):
                    if mt % 2 == 0:
                        wt2 = wpool.tile([128, FT, 256], BF16, tag="wqk")
                        nc.sync.dma_start(
                            out=wt2,
                            in_=wqk[:, 128 * mt : 128 * (mt + 2)].rearrange(
                                "(kt p) m -> p kt m", p=128
                            ),
                        )
                        wts[mt // 2] = wt2
                    wt = wts[mt // 2]
                    mo = 128 * (mt % 2)
                    ps = psp.tile([128, R], F32, tag="q_ps")
                    for kt in range(FT):
                        nc.tensor.matmul(
                            ps, lhsT=wt[:, kt, mo : mo + 128], rhs=xnT[:, kt, :],
                            start=(kt == 0), stop=(kt == FT - 1),
                        )
                    nc.vector.tensor_scalar_add(
                        out=qT_sb[:, mt, :], in0=ps,
                        scalar1=bqk_sb[:, mt : mt + 1],
                    )

            # ================= attention =================
            # k-major scores; AV with V_aug stationary so each (head, slot)
            # matmul emits y^T [65, 512] directly (row 64 = softmax denom).
            # pass 1: own-chunk (diagonal) -> yown_sb partials in SBUF (no
            # AllGather dependency); pass 2: gathered slots accumulate in PSUM,
            # then merge + normalize into feature-major yT (reusing xnT).
            with tc.tile_pool(name="kstage", bufs=1) as kpool, \
                 tc.tile_pool(name="vstage", bufs=1) as vpool, \
                 tc.tile_pool(name="attn", bufs=3) as apool, \
                 tc.tile_pool(name="expS", bufs=3) as epool, \
                 tc.tile_pool(name="aheadp", bufs=8) as aheadp:

                yown_sb = apool.tile([128, H, R], BF16, tag="yown")

                # ---- pass 1: own-chunk ragged (triangle) attention; runs while
                # the AllGathers fly. Slot s only covers queries q >= 128*s.
                with tc.tile_pool(name="sTo_ps", bufs=2, space="PSUM") as sto_ps, \
                     tc.tile_pool(name="yo_ps", bufs=2, space="PSUM") as yo_psp:
                    for h in range(H):
                        po, fi = 64 * (h % 2), h // 2
                        sT = sto_ps.tile([128, OWN_W], F32, tag="sTo")
                        for s in range(QT):
                            nc.tensor.matmul(
                                sT[:, OWN_OFF[s] : OWN_OFF[s] + OWN_N[s]],
                                lhsT=kT_sb[po : po + 64, fi, 128 * s : 128 * (s + 1)],
                                rhs=qT_sb[po : po + 64, fi, 128 * s :],
                                start=True, stop=True,
                            )
                        ex = epool.tile([128, OWN_W], BF16, tag="expO")
                        nc.scalar.activation(
                            out=ex, in_=sT,
                            func=mybir.ActivationFunctionType.Exp, scale=0.125,
                        )
                        for s in range(QT):
                            # diagonal block: keep where (q - 128*s) - p >= 0
                            nc.gpsimd.affine_select(
                                out=ex[:, OWN_OFF[s] : OWN_OFF[s] + 128],
                                in_=ex[:, OWN_OFF[s] : OWN_OFF[s] + 128],
                                compare_op=mybir.AluOpType.is_ge,
                                fill=0.0,
                                base=0,
                                pattern=[[1, 128]],
                                channel_multiplier=-1,
                            )
                        y_ps = yo_psp.tile([65, R], F32, tag="yh")
                        for s in range(QT):
                            nc.tensor.matmul(
                                y_ps[:, 128 * s :],
                                lhsT=vaug_sb[:, s, 65 * h : 65 * h + 65],
                                rhs=ex[:, OWN_OFF[s] : OWN_OFF[s] + OWN_N[s]],
                                start=(s == 0), stop=(s == QT - 1),
                            )
                        nc.vector.tensor_copy(out=yown_sb[0:65, h, :], in_=y_ps)

                # ---- stage gathered K/V in consumption order: K half a
                # (heads 0-7), then V, then K half b (heads 8-15) ----
                kst_a, kst_b, vst = [], [], []
                for s in range(NGATH):
                    c, ct = s // 4, s % 4
                    ka_t = kpool.tile([128, 4, 128], BF16, tag=f"ksta{s}")
                    nc.sync.dma_start(
                        out=ka_t,
                        in_=kt_gath_a[HC * c : HC * (c + 1), 128 * ct : 128 * (ct + 1)]
                        .rearrange("(ft p) k -> p ft k", p=128),
                    )
                    kst_a.append(ka_t)
                for s in range(NGATH):
                    v_t = vpool.tile([128, VAW], BF16, tag=f"vst{s}")
                    nc.sync.dma_start(out=v_t, in_=v_gath[128 * s : 128 * (s + 1), :])
                    nc.gpsimd.tensor_scalar_mul(
                        out=v_t, in0=v_t, scalar1=vz_bc[:, s : s + 1]
                    )
                    vst.append(v_t)
                for s in range(NGATH):
                    c, ct = s // 4, s % 4
                    kb_t = kpool.tile([128, 4, 128], BF16, tag=f"kstb{s}")
                    nc.sync.dma_start(
                        out=kb_t,
                        in_=kt_gath_b[HC * c : HC * (c + 1), 128 * ct : 128 * (ct + 1)]
                        .rearrange("(ft p) k -> p ft k", p=128),
                    )
                    kst_b.append(kb_t)

                # ---- pass 2: gathered slots, merge with own partials, normalize ----
                with tc.tile_pool(name="sT_ps", bufs=2, space="PSUM") as sts_ps, \
                     tc.tile_pool(name="y_ps", bufs=2, space="PSUM") as y_psp, \
                     tc.tile_pool(name="rec_ps", bufs=2, space="PSUM") as rec_psp:

                    def qk_exp(g, h, tag="expS"):
                        """QK matmuls for gathered slot group g + exp."""
                        po, fi = 64 * (h % 2), h // 2
                        kh, kfi = (kst_a, fi) if fi < 4 else (kst_b, fi - 4)
                        qTh = qT_sb[po : po + 64, fi, :]
                        sT = sts_ps.tile([128, 1024], F32, tag="sT")
                        for i, s in enumerate(g):
                            nc.tensor.matmul(
                                sT[:, 512 * i : 512 * (i + 1)],
                                lhsT=kh[s][po : po + 64, kfi, :], rhs=qTh,
                                start=True, stop=True,
                            )
                        pl = epool if tag == "expS" else aheadp
                        ex = pl.tile([128, 1024], BF16, tag=tag)
                        nc.scalar.activation(
                            out=ex, in_=sT,
                            func=mybir.ActivationFunctionType.Exp, scale=0.125,
                        )
                        return ex

                    # For the first NH_AHEAD heads, run QK+exp before any
                    # V-dependent work to cover the V AllGather tail.
                    NH_AHEAD = 2
                    ahead = {}
                    for h in range(NH_AHEAD):
                        exs = []
                        for g in GATH_GROUPS:
                            ex = qk_exp(g, h, tag=f"exh{h}")
                            exs.append(ex)
                        ahead[h] = exs
                    for h in range(H):
                        y_ps = y_psp.tile([65, R], F32, tag="yh")
                        for gi, g in enumerate(GATH_GROUPS):
                            if h in ahead:
                                ex = ahead[h][gi]
                            else:
                                ex = qk_exp(g, h)
                            for i, s in enumerate(g):
                                nc.tensor.matmul(
                                    y_ps,
                                    lhsT=vst[s][:, 65 * h : 65 * h + 65],
                                    rhs=ex[:, 512 * i : 512 * (i + 1)],
                                    start=(gi == 0 and i == 0),
                                    stop=(gi == len(GATH_GROUPS) - 1 and i == len(g) - 1),
                                )
                        # merge own+gathered, divide rows 0..63 by denom row 64
                        ytot = apool.tile([65, R], F32, tag="ytot")
                        nc.vector.tensor_tensor(
                            out=ytot, in0=y_ps, in1=yown_sb[0:65, h, :],
                            op=mybir.AluOpType.add,
                        )
                        rec = apool.tile([1, R], BF16, tag="rec")
                        with nc.allow_low_precision(reason="softmax denom recip bf16"):
                            nc.vector.reciprocal(out=rec, in_=ytot[64:65, :])
                        rec_bc = rec_psp.tile([64, R], F32, tag="rec_bc")
                        nc.tensor.matmul(rec_bc, lhsT=ones64, rhs=rec,
                                         start=True, stop=True)
                        po, fi = 64 * (h % 2), h // 2
                        nc.vector.tensor_tensor(
                            out=xnT[po : po + 64, fi, :], in0=ytot[0:64, :], in1=rec_bc,
                            op=mybir.AluOpType.mult,
                        )

            # ================= attn proj + resid1 =================
            yT = xnT  # attention wrote y^T feature-major into xnT's slots
            with tc.tile_pool(name="wap_pool", bufs=1) as wpool, \
                 tc.tile_pool(name="ap_ps", bufs=2, space="PSUM") as psp:
                wap_sb = wpool.tile([128, FT, C], BF16)
                for kt in range(FT):
                    nc.sync.dma_start(
                        out=wap_sb[:, kt, :],
                        in_=wap[128 * kt : 128 * (kt + 1), :],
                    )
                bap_bc = bcast_tile(bap, C, name="bap", pool=wpool)
                for qt in range(QT):
                    ps = psp.tile([128, C], F32, tag="ap_ps")
                    for kt in range(FT):
                        for half in range(2):
                            nc.tensor.matmul(
                                ps[:, 512 * half : 512 * (half + 1)],
                                lhsT=yT[:, kt, 128 * qt : 128 * (qt + 1)],
                                rhs=wap_sb[:, kt, 512 * half : 512 * (half + 1)],
                                start=(kt == 0), stop=(kt == FT - 1),
                            )
                    nc.vector.tensor_tensor(
                        out=x_sb[:, qt, :], in0=x_sb[:, qt, :], in1=ps,
                        op=mybir.AluOpType.add,
                    )
                    nc.vector.tensor_tensor(
                        out=x_sb[:, qt, :], in0=x_sb[:, qt, :], in1=bap_bc,
                        op=mybir.AluOpType.add,
                    )

            # ================= LN2 + transpose =================
            mlp_ctx = ExitStack()
            xn2T = mlp_ctx.enter_context(
                tc.tile_pool(name="xn2t", bufs=1)
            ).tile([128, FT, R], BF16, tag="xn2T")
            hT_sb = mlp_ctx.enter_context(
                tc.tile_pool(name="mlp", bufs=1)
            ).tile([128, 32, R], BF16, tag="hT")
            with tc.tile_pool(name="ln2", bufs=3) as pool, \
                 tc.tile_pool(name="tr3_ps", bufs=2, space="PSUM") as psp:
                ln2gT = pool.tile([128, FT], F32, tag="ln2gT")
                nc.sync.dma_start(out=ln2gT, in_=ln2g[:].rearrange("(ft p) -> p ft", p=128))
                ln2bT = pool.tile([128, FT], F32, tag="ln2bT")
                nc.sync.dma_start(out=ln2bT, in_=ln2b[:].rearrange("(ft p) -> p ft", p=128))
                for qt in range(QT):
                    xn = layernorm(pool, x_sb[:, qt, :])
                    transpose_qt(pool, psp, xn, xn2T, qt, gT=ln2gT, bT=ln2bT)

            # ================= MLP fc + gelu =================
            with tc.tile_pool(name="wfc_pool", bufs=3) as wpool, \
                 tc.tile_pool(name="fc_ps", bufs=4, space="PSUM") as psp:
                cur_wfc = [None]
                for mt in range(32):
                    if mt % 2 == 0:
                        wt2 = wpool.tile([128, FT, 256], BF16, tag="wfc")
                        nc.sync.dma_start(
                            out=wt2,
                            in_=wfc[:, 128 * mt : 128 * (mt + 2)].rearrange(
                                "(kt p) m -> p kt m", p=128
                            ),
                        )
                        cur_wfc[0] = wt2
                    wt = cur_wfc[0]
                    mo = 128 * (mt % 2)
                    ps = psp.tile([128, R], F32, tag="fc_ps")
                    for kt in range(FT):
                        nc.tensor.matmul(
                            ps, lhsT=wt[:, kt, mo : mo + 128], rhs=xn2T[:, kt, :],
                            start=(kt == 0), stop=(kt == FT - 1),
                        )
                    nc.scalar.activation(
                        out=hT_sb[:, mt, :], in_=ps,
                        func=mybir.ActivationFunctionType.Gelu_apprx_tanh,
                        bias=bfc_sb[:, mt : mt + 1], scale=1.0,
                    )

            # ================= MLP proj (token-major) + resid2 + out =================
            # hT slice stationary, wmp moving -> out [q, C] token-major; the
            # residual add reads x_sb directly (no x2 transpose needed).
            with tc.tile_pool(name="wmp_pool", bufs=3) as wpool, \
                 tc.tile_pool(name="outp", bufs=3) as opool, \
                 tc.tile_pool(name="mp_ps", bufs=1, space="PSUM") as psp:
                bmp_bc = bcast_tile(bmp, C, name="bmp", pool=wpool)
                for wave in ((0, 1), (2, 3)):
                    pss = {}
                    for qt in wave:
                        mp_ps_t = psp.tile([128, C], F32, tag=f"mp_ps{qt}")
                        pss[qt] = mp_ps_t
                    for kt in range(32):
                        wt = wpool.tile([128, C], BF16, tag=f"wmp{wave[0]}")
                        nc.sync.dma_start(
                            out=wt,
                            in_=wmp[128 * kt : 128 * (kt + 1), :],
                        )
                        for qt in wave:
                            for half in range(2):
                                nc.tensor.matmul(
                                    pss[qt][:, 512 * half : 512 * (half + 1)],
                                    lhsT=hT_sb[:, kt, 128 * qt : 128 * (qt + 1)],
                                    rhs=wt[:, 512 * half : 512 * (half + 1)],
                                    start=(kt == 0), stop=(kt == 31),
                                )
                    for qt in wave:
                        o = opool.tile([128, C], F32, tag="out")
                        nc.vector.tensor_tensor(
                            out=o, in0=pss[qt], in1=x_sb[:, qt, :], op=mybir.AluOpType.add
                        )
                        nc.vector.tensor_tensor(
                            out=o, in0=o, in1=bmp_bc, op=mybir.AluOpType.add
                        )
                        nc.sync.dma_start(
                            out=out[:].rearrange("(qt p) c -> p qt c", p=128)[:, qt, :], in_=o
                        )
            mlp_ctx.close()

        for it in range(iters):
            body(it)

    nc.compile()
    return nc


def make_core_inputs(full):
    """full: dict of np arrays as in reference.setup_inputs(). Returns list of 8 in_maps."""
    import ml_dtypes

    bf = lambda a: np.asarray(a, np.float32).astype(ml_dtypes.bfloat16)
    f32 = lambda a: np.ascontiguousarray(np.asarray(a, np.float32))
    W_attn = np.asarray(full["W_attn"], np.float32)
    b_attn = np.asarray(full["b_attn"], np.float32)
    shared = {
        "w_qk": bf(W_attn[:, : 2 * C]),
        "b_qk": f32(b_attn[: 2 * C]),
        "w_v": bf(W_attn[:, 2 * C :]),
        "b_v": f32(b_attn[2 * C :]),
        "w_ap": bf(full["W_ap"]),
        "b_ap": f32(full["b_ap"]),
        "w_fc": bf(full["W_fc"]),
        "b_fc": f32(full["b_fc"]),
        "w_mp": bf(full["W_mp"]),
        "b_mp": f32(full["b_mp"]),
        "ln1_g": f32(full["ln1_g"]),
        "ln1_b": f32(full["ln1_b"]),
        "ln2_g": f32(full["ln2_g"]),
        "ln2_b": f32(full["ln2_b"]),
    }
    x = np.asarray(full["x"], np.float32)
    in_maps = []
    for core in range(8):
        b, j = core // 4, core % 4
        m = dict(shared)
        m["x"] = np.ascontiguousarray(x[b, R * j : R * (j + 1), :])
        m["vzero"] = (np.arange(NGATH) < 4 * j).astype(np.float32)
        in_maps.append(m)
    return in_maps


def assemble(results):
    outs = [np.asarray(results[c]["out"]) for c in range(8)]
    return np.stack(
        [np.concatenate(outs[0:4], axis=0), np.concatenate(outs[4:8], axis=0)]
    )


_NC_CACHE = []


def kernel(**inputs):
    import time
    import numpy as np
    from concourse.bass_utils import run_bass_kernel_spmd

    if not _NC_CACHE:
        _NC_CACHE.append(build_nc())
    nc = _NC_CACHE[0]
    in_maps = make_core_inputs(inputs)
    last = None
    for attempt in range(3):
        try:
            res = run_bass_kernel_spmd(nc, in_maps, list(range(8)))
            return assemble(res.results).astype(np.float32)
        except Exception as e:  # transient axon mesh desync -> retry
            last = e
            time.sleep(5.0)
    raise last
